# revision 17
# baseline (speedup 1.0000x reference)
"""Trainium2 Bass kernel for nn_ClauseInferModule (gnn_message_passing).

out[c, b, g] = sum_s prod_l x[b, I[c, g, s, l]],  B=16 G=16384 C=8 S=8 L=4.

Clause-per-core. The gather runs on GPSIMD ap_gather (measured ~27ns per
index per Q7 core, RD_CMD-latency-bound; all 8 Q7 cores work in parallel on
their own 16-partition group). v7 over the session baseline:
  - x is staged once (1MB, fp32r chunks) and fanned out 16->128 partitions
    by a one-hot fp32r matmul on the otherwise-idle PE with ACT/DVE PSUM
    drains (~32us vs ~59us for DMA-replicated staging; fp32r rounding puts
    the final rel err at ~5e-4, far under the 2e-2 gate),
  - the ap_gather library is loaded explicitly at t=0 and warmed up with a
    dummy 16-index gather during staging,
  - the output is dumped as the raw [128, 2048] acc tile in one DMA and
    re-laid-out on the host,
  - the last substitution's gather is split in half so the final DVE
    product chain hides under the second half.
"""
import os
import sys
import numpy as np

sys.path.insert(0, "/opt/trn_rl_repo")

import concourse.bacc as bacc
import concourse.tile as tile
from concourse import mybir
from concourse.bass_utils import run_bass_kernel_spmd
from concourse.library_config import ap_gather as ap_gather_lib

B, G = 16, 16384
C, S, L = 8, 8, 4
NIDX = 8192          # gathers per core group per full ap_gather call
GCHUNK = G // 8      # 2048 target atoms per core group
ICOL = NIDX // 16    # 512 idx columns per call in the wrapped int16 feed

_compiled = None
last_exec_time_ns = None


def _build():
    nc = bacc.Bacc("TRN2", target_bir_lowering=False, debug=False)
    x1_d = nc.dram_tensor("x1", [B, G], mybir.dt.float32r, kind="ExternalInput")
    e_d = nc.dram_tensor("eye", [B, 128], mybir.dt.float32r, kind="ExternalInput")
    idx_d = nc.dram_tensor("idx", [128, S * ICOL], mybir.dt.int16,
                           kind="ExternalInput")
    out_d = nc.dram_tensor("out", [128, GCHUNK], mybir.dt.float32,
                           kind="ExternalOutput")

    with tile.TileContext(nc) as tc:
        with tc.tile_pool(name="xq", bufs=1) as xq, \
             tc.tile_pool(name="ip", bufs=3) as ip, \
             tc.tile_pool(name="gp", bufs=2) as gp, \
             tc.tile_pool(name="wp", bufs=1) as wp, \
             tc.tile_pool(name="aq", bufs=1) as aq, \
             tc.psum_pool(name="pp", bufs=2) as pp:
            nc.gpsimd.load_library(ap_gather_lib)
            x_tile = xq.tile([128, G], mybir.dt.float32)
            it_tiles = {}
            for s0 in range(2):
                it = ip.tile([128, ICOL], mybir.dt.int16, tag="it",
                             name=f"itp{s0}")
                nc.sync.dma_start(
                    out=it[:, :], in_=idx_d[:, s0 * ICOL:(s0 + 1) * ICOL])
                it_tiles[s0] = it
            # Stage x in fp32r chunks and fan out 16->128 partitions with a
            # one-hot fp32r matmul (PE): out[p, f] = x1[p % 16, f]. The
            # matmul reads only the small xs chunk tiles (never x_tile), so
            # the BIR fp32r producer rule is satisfied by the chunk DMAs.
            e_t = xq.tile([B, 128], mybir.dt.float32r, name="e_t")
            nc.scalar.dma_start(out=e_t[:, :], in_=e_d[:, :])
            for r in range(8):
                xs = xq.tile([B, 2048], mybir.dt.float32r, tag="xs",
                             name=f"xs{r}", bufs=2)
                nc.sync.dma_start(out=xs[:, :],
                                  in_=x1_d[:, r * 2048:(r + 1) * 2048])
                ps = pp.tile([128, 2048], mybir.dt.float32, tag="ps",
                             name=f"ps{r}")
                for j in range(4):
                    nc.tensor.matmul(
                        ps[:, j * 512:(j + 1) * 512], e_t[:, :],
                        xs[:, j * 512:(j + 1) * 512])
                lo = r * 2048
                nc.scalar.copy(x_tile[:, lo:lo + 1024], ps[:, 0:1024])
                nc.vector.tensor_copy(x_tile[:, lo + 1024:lo + 2048],
                                      ps[:, 1024:2048])
            # warm-up gather: pays the ~2us first-invocation library IRAM
            # pull during the staging window instead of in the first call
            wt = xq.tile([128, 16], mybir.dt.float32, name="warm_t")
            wi = xq.tile([128, 1], mybir.dt.int16, name="warm_i")
            wg = xq.tile([128, 16], mybir.dt.float32, name="warm_g")
            nc.gpsimd.memset(wt[:, :], 0.0)
            nc.gpsimd.memset(wi[:, :], 0)
            nc.gpsimd.ap_gather(wg[:, :], wt[:, :], wi[:, :],
                                channels=128, num_elems=16, d=1, num_idxs=16)
            acc = aq.tile([128, GCHUNK], mybir.dt.float32)

            def gather(sl, tag):
                """One ap_gather over idx columns sl (slice into S*ICOL)."""
                ncol = sl.stop - sl.start
                if tag in it_tiles:
                    it = it_tiles[tag]
                else:
                    it = ip.tile([128, ncol], mybir.dt.int16, tag="it",
                                 name=f"it{tag}")
                    nc.sync.dma_start(out=it[:, :], in_=idx_d[:, sl])
                g = gp.tile([128, ncol * 16], mybir.dt.float32, tag="g",
                            name=f"g{tag}")
                nc.gpsimd.ap_gather(g[:, :], x_tile[:, :], it[:, :],
                                    channels=128, num_elems=G, d=1,
                                    num_idxs=ncol * 16)
                return g

            def reduce_l(g, colrange, first):
                """tm = prod_l g over GCHUNK cols; acc (+)= tm."""
                w = colrange.stop - colrange.start

                def A(l):
                    return g[:, (l * GCHUNK + colrange.start):
                             (l * GCHUNK + colrange.stop)]

                tm1 = wp.tile([128, GCHUNK], mybir.dt.float32, tag="tm1",
                              name="tm1")
                tm2 = wp.tile([128, GCHUNK], mybir.dt.float32, tag="tm2",
                              name="tm2")
                o1 = tm1[:, colrange.start:colrange.start + w]
                o2 = tm2[:, colrange.start:colrange.start + w]
                nc.vector.tensor_mul(o1, A(0), A(1))
                nc.vector.tensor_mul(o2, A(2), A(3))
                ao = acc[:, colrange.start:colrange.stop]
                if first:
                    nc.vector.tensor_mul(ao, o1, o2)
                else:
                    tm3 = wp.tile([128, GCHUNK], mybir.dt.float32, tag="tm3",
                                  name="tm3")
                    o3 = tm3[:, colrange.start:colrange.start + w]
                    nc.vector.tensor_mul(o3, o1, o2)
                    nc.vector.tensor_add(ao, ao, o3)

            for s in range(S - 1):
                g = gather(slice(s * ICOL, (s + 1) * ICOL), s)
                reduce_l(g, slice(0, GCHUNK), s == 0)
            # Last substitution: split so the final DVE chain and the output
            # DMA of the first half hide under the second half's gather.
            # Stream layout per call is l-major (i = l*2048 + w), so a half
            # call must cover all 4 l's for half the atoms. Host packs the
            # last call's idx columns as two self-contained half streams.
            s = S - 1
            base = s * ICOL
            NQ4 = 4
            qg = GCHUNK // NQ4
            qcol = ICOL // NQ4

            def reduce_quarter(g, q, tag):
                def A(l):
                    return g[:, l * qg:(l + 1) * qg]

                tm1 = wp.tile([128, GCHUNK], mybir.dt.float32, tag="tm1",
                              name=f"h1{tag}")
                tm2 = wp.tile([128, GCHUNK], mybir.dt.float32, tag="tm2",
                              name=f"h2{tag}")
                tm3 = wp.tile([128, GCHUNK], mybir.dt.float32, tag="tm3",
                              name=f"h3{tag}")
                o1 = tm1[:, 0:qg]
                o2 = tm2[:, 0:qg]
                o3 = tm3[:, 0:qg]
                nc.vector.tensor_mul(o1, A(0), A(1))
                nc.vector.tensor_mul(o2, A(2), A(3))
                nc.vector.tensor_mul(o3, o1, o2)
                lo = q * qg
                ao = acc[:, lo:lo + qg]
                nc.vector.tensor_add(ao, ao, o3)
                nc.sync.dma_start(out=out_d[:, lo:lo + qg], in_=ao)

            for q in range(NQ4):
                gq = gather(slice(base + q * qcol, base + (q + 1) * qcol),
                            f"7q{q}")
                reduce_quarter(gq, q, f"q{q}")
    nc.compile()
    return nc


def _prep_idx(I: np.ndarray) -> np.ndarray:
    """[C, G, S, L] int64 -> [C, 128, S*512] int16 wrapped ap_gather feed.

    Calls s=0..6: group k stream position i = l*2048 + w holds
    I[c, k*2048 + w, s, l]. Call s=7 is split into two half streams
    (atoms [0,1024) then [1024,2048) of each group's chunk), each with
    i = l*1024 + w.
    """
    T = I.astype(np.int16).reshape(C, 8, GCHUNK, S, L)     # [c,k,w,s,l]
    U = T.transpose(0, 3, 1, 4, 2)                         # [c,s,k,l,w]
    full = U[:, :S - 1].reshape(C, S - 1, 8, NIDX)         # i = l*2048 + w

    def wrap(V):
        # V: [C, n, 8, m] stream -> [C, n, 8, m/16, 16] -> [c, k, 16, n, m/16]
        n, m = V.shape[1], V.shape[3]
        W = V.reshape(C, n, 8, m // 16, 16).transpose(0, 2, 4, 1, 3)
        return W.reshape(C, 128, n * (m // 16))

    parts = [wrap(full)]                                   # [C, 128, 7*512]
    NQ4 = 4
    qg = GCHUNK // NQ4
    for q in range(NQ4):
        Vq = U[:, S - 1, :, :, q * qg:(q + 1) * qg].reshape(C, 1, 8, L * qg)
        parts.append(wrap(Vq))                             # [C, 128, 128]
    return np.ascontiguousarray(np.concatenate(parts, axis=2))


def kernel(x: np.ndarray, I: np.ndarray) -> np.ndarray:
    global _compiled, last_exec_time_ns
    if _compiled is None:
        _compiled = _build()
    nc = _compiled

    x = np.ascontiguousarray(np.asarray(x), dtype=np.float32)
    eye = np.zeros((B, 128), dtype=np.float32)
    for p in range(128):
        eye[p % B, p] = 1.0
    idx_feed = _prep_idx(np.asarray(I))

    in_maps = [{"x1": x, "eye": eye, "idx": idx_feed[c]} for c in range(C)]
    kwargs = {}
    if os.environ.get("KERNEL_TRACE") == "1":
        kwargs = {"trace": True, "trace_cores": list(range(C))}
    res = run_bass_kernel_spmd(nc, in_maps, core_ids=list(range(C)), **kwargs)
    last_exec_time_ns = res.exec_time_ns
    # out[16k + b, w] = out[c, b, k*2048 + w]
    raw = np.stack([res.results[c]["out"] for c in range(C)], axis=0)
    out = raw.reshape(C, 8, B, GCHUNK).transpose(0, 2, 1, 3).reshape(C, B, G)
    return np.ascontiguousarray(out, dtype=np.float32)


if __name__ == "__main__":
    rng = np.random.default_rng(0)
    x = rng.random((B, G), dtype=np.float32)
    I = rng.integers(0, G, size=(C, G, S, L)).astype(np.int64)
    out = kernel(x=x, I=I)
    gathered = x[:, I]
    expect = np.moveaxis(np.sum(np.prod(gathered, axis=-1), axis=-1), 0, 1)
    err = np.abs(out - expect).max() / np.abs(expect).max()
    print("max rel err:", err)
